# revision 30
# baseline (speedup 1.0000x reference)
"""Trainium2 Bass kernel for a BFP-quantized ResNet BasicBlock (inference).

Computes, per image (NCHW, C=128, H=W=56):
    out = relu( bn2( conv3x3( q( relu(bn1( conv3x3(q(x), q(w1)) )) ), q(w2)) ) + x )
where q() is HBFP block-floating-point quantization: blocks of 64 contiguous
values (in flat row-major order) share a power-of-2 scale 2^(floor(log2(max|x|))-7),
mantissas rounded (RNE) to 8 signed bits and clamped to +-127.

Key facts exploited:
  * Quantized values are (int in [-127,127]) * 2^k  -> exactly representable in
    bf16, so convs run on the PE at bf16 speed with zero extra error.
  * floor(log2(m)) for normal floats == exponent-field extraction (bitwise ops).
  * RNE rounding == (t + 1.5*2^23) - 1.5*2^23 in fp32 (one dual-op tensor_scalar).
  * clip(round(t)) == round(clamp(t, -127.4, 127.4)) elementwise.
  * conv3x3 = 9 accumulated matmuls (C_in=128 on partitions) over a zero-padded
    58-pitch image layout, fully contiguous rhs slices of 464 columns (8 rows).

Scheduling notes (v4):
  * The quantized image is inserted into the padded conv layout by a ScalarE
    strided copy, NOT a DMA (a 58-pitch DMA fragments into 112B packets that
    swamp all 16 DMA engines and inflate every DVE op via SBUF contention).
  * GpSimd shares its only SBUF port pair with DVE's 2-port modes
    (tensor_scalar/copy/cast); whoever issues first holds an exclusive lock.
    So GpSimd gets only tensor_tensor work (scale-backs, residual adds), split
    into halves so a V 2-port op never blocks behind a long G op, and emitted
    so G occupancy overlaps V's single-port phases (tensor_tensor/reduce never
    contend).
  * Weight transposes run on the DMA crossbar (dma_start_transpose) at startup:
    no PSUM round-trip, no PE/ScalarE involvement.
  * Stages are pipelined ~2 images deep; iter k emits conv2(k) | quant2(k+1) |
    final(k-1) | conv1(k+2) | quant1(k+3) | load_x(k+4).  The last image's
    residual+relu+store is fused per-chunk into conv2's eviction to shorten the
    drain tail.

Sharding: data-parallel over batch N=64 -> 8 images per NeuronCore, weights and
BN constants replicated. All 8 cores run the same NEFF (SPMD).
"""

import os

os.environ.setdefault("MYCRO_LOCAL_CACHE", "1")

from contextlib import ExitStack
from functools import lru_cache

import numpy as np

import concourse.bass as bass
import concourse.tile as tile
from concourse import bacc, mybir
from concourse.bass_utils import run_bass_kernel_spmd
from concourse.masks import make_identity

P = 128
H = W = 56
HWF = H * W            # 3136 flat pixels per channel
NBX = HWF // 64        # 49 BFP blocks per channel image
WLEN = 128 * 9         # 1152 flat weight row per output channel
NBW = WLEN // 64       # 18 BFP blocks per weight row
PITCH = W + 2          # 58 padded row pitch
PADLEN = PITCH * PITCH + 2  # 3366: [1 pre-pad][58x58 padded image][1 post-pad]
NCHUNK = 7             # 8-row output chunks per image
CHF = 8 * PITCH        # 464 matmul free dim per chunk
CROUND = 12582912.0    # 1.5 * 2**23  (RNE magic constant)
CLIPV = 127.4
EXPMASK = 0x7F800000
BIAS7 = 7 << 23
C254 = 254 << 23
EGUARD = 50 << 23      # exponent field of 1e-23 (reference's zero-guard)
BN_EPS = 1e-5

F32 = mybir.dt.float32
BF16 = mybir.dt.bfloat16
I32 = mybir.dt.int32
ALU = mybir.AluOpType
ACTF = mybir.ActivationFunctionType
AX = mybir.AxisListType

N_CORES = 8
NIMG = 8   # images per core
NXRAW = 4  # x buffers in flight
HB = 25    # block split point for halved G ops (25+24 = 49)


def _interior(pad_tile):
    """[P, 56, 56] strided view (pitch 58) of the padded tile's interior."""
    base = 1 + PITCH + 1  # (h=0, w=0) -> index 1 + (h+1)*58 + (w+1)
    v = pad_tile[:, base : base + H * PITCH]
    return v.rearrange("p (h w) -> p h w", w=PITCH)[:, :, :W]


def build_nc(nimg=NIMG):
    nc = bacc.Bacc("TRN2", target_bir_lowering=False, debug=False,
                   enable_asserts=False)

    x_d = nc.dram_tensor("x", [nimg, P, H, W], F32, kind="ExternalInput").ap()
    w1_d = nc.dram_tensor("w1", [P, P, 3, 3], F32, kind="ExternalInput").ap()
    w2_d = nc.dram_tensor("w2", [P, P, 3, 3], F32, kind="ExternalInput").ap()
    bn_d = {
        name: nc.dram_tensor(name, [P], F32, kind="ExternalInput").ap()
        for name in ("gamma1", "beta1", "mean1", "var1",
                     "gamma2", "beta2", "mean2", "var2")
    }
    out_d = nc.dram_tensor("out", [nimg, P, H, W], F32, kind="ExternalOutput").ap()

    with tile.TileContext(nc) as tc, ExitStack() as ctx:
        const = ctx.enter_context(tc.tile_pool(name="const", bufs=1))
        small = ctx.enter_context(tc.tile_pool(name="small", bufs=2))
        xraw_p = ctx.enter_context(tc.tile_pool(name="xraw", bufs=NXRAW))

        xraws = [None] * nimg

        def load_x(n):
            xraw = xraw_p.tile([P, HWF], F32, tag="xraw", name=f"xraw{n}")
            xraws[n] = xraw
            nc.sync.dma_start(xraw[:], x_d[n].rearrange("c h w -> c (h w)"))

        def _emit_scale_bits(src3, nb, tag, relu_fused=False, eng=None):
            """Per-block scale/rscale from src3 (f32 [P, nb, 64]).
            relu_fused: src is PRE-relu; the block max of relu(src) is
            max(max(src), 0) -- no abs in the reduce, then clamp at 0."""
            eng = eng or nc.vector
            bm = small.tile([P, nb], F32, tag=f"bm_{tag}")
            sb = small.tile([P, nb], I32, tag=f"sb_{tag}")
            rb = small.tile([P, nb], I32, tag=f"rb_{tag}")
            sc = small.tile([P, nb], BF16, tag=f"sc_{tag}")
            # free-dim reduce is VectorE-only
            nc.vector.tensor_reduce(
                out=bm[:], in_=src3, axis=AX.X, op=ALU.max,
                apply_absolute_value=not relu_fused,
            )
            if relu_fused:
                eng.tensor_scalar(bm[:], bm[:], 0.0, None, ALU.max)
            # scale bits = max(exponent field, expfield(1e-23)) - (7 << 23)
            eng.tensor_scalar(sb[:], bm[:].bitcast(I32), EXPMASK, None,
                              ALU.bitwise_and)
            eng.tensor_scalar(sb[:], sb[:], EGUARD, BIAS7,
                              ALU.max, ALU.subtract)
            # rscale bits = (254 << 23) - scale_bits -> rscale = 2^(7-e)
            eng.tensor_scalar(rb[:], sb[:], C254, -1,
                              ALU.subtract, ALU.mult)
            eng.tensor_copy(sc[:], sb[:].bitcast(F32))
            return rb, sc

        # ---- DMAs: x0 then w1 head the startup critical path ----
        if True:
            setup = ctx.enter_context(tc.tile_pool(name="setup", bufs=1))
            wraws = []
            for wi, w_d in enumerate((w1_d, w2_d)):
                wraw = setup.tile([P, WLEN], F32, tag=f"wraw{wi}")
                wraws.append(wraw)
            load_x(0)
            nc.sync.dma_start(wraws[0][:],
                              w1_d.rearrange("o i kh kw -> o (i kh kw)"))
            for n in range(1, min(4, nimg)):
                load_x(n)
            bnc = {}
            for name in ("gamma1", "beta1", "mean1", "var1",
                         "gamma2", "beta2", "mean2", "var2"):
                t = setup.tile([P, 1], F32, tag=f"bn_{name}")
                nc.sync.dma_start(t[:], bn_d[name][:, None])
                bnc[name] = t
            nc.sync.dma_start(wraws[1][:],
                              w2_d.rearrange("o i kh kw -> o (i kh kw)"))

            zero_b = const.tile([P, 1], F32, tag="zero_b")
            nc.vector.memset(zero_b[:], 0.0)
            eps_b = const.tile([P, 1], F32, tag="eps_b")
            nc.vector.memset(eps_b[:], BN_EPS)

            # padded conv-input tiles; zeroed once on GpSimd (idle at startup)
            pads = ctx.enter_context(tc.tile_pool(name="pads", bufs=1))
            tmp_p = ctx.enter_context(tc.tile_pool(name="tmp", bufs=2))
            u_p = ctx.enter_context(tc.tile_pool(name="u", bufs=2))
            mid_p = ctx.enter_context(tc.tile_pool(name="mid", bufs=2))
            t2_p = ctx.enter_context(tc.tile_pool(name="t2", bufs=2))
            xq_pads = [pads.tile([P, PADLEN], BF16, tag=f"xqp{i}", name=f"xqp{i}")
                       for i in range(2)]
            mq_pads = [pads.tile([P, PADLEN], BF16, tag=f"mqp{i}", name=f"mqp{i}")
                       for i in range(2)]
            # bf16 x in padded layout: conv2's residual-matmul operand
            xb_pads = [pads.tile([P, PADLEN], BF16, tag=f"xbp{i}", name=f"xbp{i}")
                       for i in range(2)]
            for t in (*xq_pads, *mq_pads, *xb_pads):
                nc.gpsimd.memset(t[:], 0.0)

            def _setup_w(wi):
                """Quantize weight wi (GpSimd -- VectorE is busy with the
                first image's quant at startup) + transpose via DMA xbar."""
                wraw = wraws[wi]
                wsrc3 = wraw[:].rearrange("p (b e) -> p b e", e=64)
                rb, sc = _emit_scale_bits(wsrc3, NBW, f"w{wi}")
                wt_t = setup.tile([P, WLEN], F32, tag="wt")
                wt3 = wt_t[:].rearrange("p (b e) -> p b e", e=64)
                rsc = rb[:].bitcast(F32)[:, :, None].to_broadcast((P, NBW, 64))
                nc.vector.tensor_tensor(wt3, wsrc3, rsc, ALU.mult)
                nc.vector.tensor_scalar(wt_t[:], wt_t[:], CLIPV, -CLIPV,
                                        ALU.min, ALU.max)
                wq = setup.tile([P, WLEN], BF16, tag=f"wq{wi}")
                nc.vector.tensor_scalar(wq[:], wt_t[:], CROUND, CROUND,
                                        ALU.add, ALU.subtract)
                scb = sc[:][:, :, None].to_broadcast((P, NBW, 64))
                wq3 = wq[:].rearrange("p (b e) -> p b e", e=64)
                nc.vector.tensor_tensor(wq3, wq3, scb, ALU.mult)
                # regroup k-major (contiguous per k), then per-offset lhsT
                # tiles w[k][i, o] = wq[o, i*9+k] via the DMA crossbar
                wq_r = setup.tile([P, WLEN], BF16, tag="wqr")
                nc.vector.tensor_copy(
                    wq_r[:].rearrange("p (k i) -> p k i", k=9),
                    wq[:].rearrange("p (i k) -> p k i", k=9))
                wk = []
                for k in range(9):
                    wt = const.tile([P, P], BF16, tag=f"w{wi}k{k}")
                    nc.sync.dma_start_transpose(wt[:], wq_r[:, k * P:(k + 1) * P])
                    wk.append(wt)
                return wk

            def _setup_bn():
                invb = []
                for i in ("1", "2"):
                    s = setup.tile([P, 1], F32, tag=f"sd{i}")
                    nc.scalar.activation(s[:], bnc[f"var{i}"][:], ACTF.Sqrt,
                                         bias=eps_b[:])
                    r = setup.tile([P, 1], F32, tag=f"rs{i}")
                    nc.vector.reciprocal(r[:], s[:])
                    inv = const.tile([P, 1], F32, tag=f"inv{i}")
                    nc.vector.tensor_tensor(inv[:], bnc[f"gamma{i}"][:], r[:],
                                            ALU.mult)
                    mi = setup.tile([P, 1], F32, tag=f"mi{i}")
                    nc.vector.tensor_tensor(mi[:], bnc[f"mean{i}"][:], inv[:],
                                            ALU.mult)
                    b = const.tile([P, 1], F32, tag=f"b{i}")
                    nc.vector.tensor_tensor(b[:], bnc[f"beta{i}"][:], mi[:],
                                            ALU.subtract)
                    invb.append((inv, b))
                # diag(1/inv2) in bf16: evict2 applies scale=inv2 to the whole
                # PSUM, so the residual matmul pre-divides x by inv2
                ident = setup.tile([P, P], BF16, tag="ident")
                make_identity(nc, ident[:])
                r2 = setup.tile([P, 1], F32, tag="r2d")
                nc.vector.reciprocal(r2[:], invb[1][0][:])
                resw = const.tile([P, P], BF16, tag="resw")
                nc.vector.tensor_tensor(
                    resw[:], ident[:], r2[:].to_broadcast((P, P)), ALU.mult)
                return invb, resw

            psum1_p = ctx.enter_context(
                tc.tile_pool(name="psum1", bufs=4, space="PSUM"))
            psum2_p = ctx.enter_context(
                tc.tile_pool(name="psum2", bufs=4, space="PSUM"))

            mids = [None] * nimg
            t2s = [None] * nimg

            def _quant_image(src, pad, tagq, nm, split=False,
                             relu_fused=False):
                """BFP-quantize src (f32 [P,3136]) into pad's interior.
                V: reduce/smalls/mult/clip/round; G: scale-back; S: strided
                insert into the padded layout.  split=True pipelines the chain
                in two halves (startup: lets conv chunks 0-2 begin early).
                relu_fused: src is pre-relu; clip to [0, CLIPV] instead of
                [-CLIPV, CLIPV] (relu commutes with the positive rescale)."""
                src3 = src[:].rearrange("p (b e) -> p b e", e=64)
                rb, sc = _emit_scale_bits(src3, NBX, tagq, relu_fused)
                t = tmp_p.tile([P, HWF], F32, tag="t", name=f"t_{nm}")
                t3 = t[:].rearrange("p (b e) -> p b e", e=64)
                rsc = rb[:].bitcast(F32)[:, :, None].to_broadcast((P, NBX, 64))
                u = u_p.tile([P, HWF], BF16, tag="u", name=f"u_{nm}")
                u3 = u[:].rearrange("p (b e) -> p b e", e=64)
                scb = sc[:][:, :, None].to_broadcast((P, NBX, 64))
                uhw = u[:].rearrange("p (h w) -> p h w", w=W)
                halves = ((0, 28), (28, NBX)) if split else ((0, NBX),)
                for b0, b1 in halves:
                    bs = slice(b0, b1)
                    es = slice(b0 * 64, b1 * 64)
                    rows = slice(b0 * 64 // W, b1 * 64 // W)
                    nc.vector.tensor_tensor(t3[:, bs], src3[:, bs], rsc[:, bs],
                                            ALU.mult)
                    nc.vector.tensor_scalar(
                        t[:, es], t[:, es], CLIPV,
                        0.0 if relu_fused else -CLIPV, ALU.min, ALU.max)
                    nc.vector.tensor_scalar(u[:, es], t[:, es], CROUND, CROUND,
                                            ALU.add, ALU.subtract)
                    nc.gpsimd.tensor_tensor(u3[:, bs], u3[:, bs], scb[:, bs],
                                            ALU.mult)
                    nc.scalar.copy(_interior(pad)[:, rows], uhw[:, rows])

            def quant1(n):
                _quant_image(xraws[n], xq_pads[n % 2], "q1", f"q1_{n}",
                             split=True)

            def quant2(n):
                # always split: conv2's chunks 0-2 depend only on half 1
                _quant_image(mids[n], mq_pads[n % 2], "q2", f"q2_{n}",
                             split=True)
                # padded bf16 copy of x: residual matmul operand for conv2
                nc.scalar.copy(_interior(xb_pads[n % 2]),
                               xraws[n][:].rearrange("p (h w) -> p h w", w=W))

            def _emit_conv(psum_pool, wk, src_pad, evict, res_pad=None):
                for c in range(NCHUNK):
                    h0 = c * 8
                    ps = psum_pool.tile([P, CHF], F32, tag="pschunk")
                    for k in range(9):
                        kh, kw = divmod(k, 3)
                        s = (h0 + kh) * PITCH + kw
                        nc.tensor.matmul(
                            ps[:], wk[k][:], src_pad[:, s : s + CHF],
                            start=(k == 0), stop=(k == 8 and res_pad is None),
                        )
                    if res_pad is not None:
                        s = (h0 + 1) * PITCH + 1  # center tap: aligned rows
                        nc.tensor.matmul(ps[:], resw[:], res_pad[:, s : s + CHF],
                                         start=False, stop=True)
                    evict(c, ps)

            def conv1(n):
                mid = mid_p.tile([P, HWF], F32, tag="mid", name=f"mid{n}")
                mids[n] = mid

                def evict1(c, ps):
                    psv = ps[:].rearrange("p (r w) -> p r w", w=PITCH)[:, :, 1:1 + W]
                    ov = mid[:, c * 448:(c + 1) * 448].rearrange(
                        "p (r w) -> p r w", w=W)
                    nc.scalar.activation(ov, psv, ACTF.Relu,
                                         bias=b1[:], scale=inv1[:])

                _emit_conv(psum1_p, w1k, xq_pads[n % 2][:], evict1)

            def conv2(n, chunk_store=False):
                """conv + residual (10th matmul) in PSUM; evict applies
                relu(inv2*psum + b2) = relu(bn2(conv) + x) and the result goes
                straight to HBM -- no separate final stage."""
                t2 = t2_p.tile([P, HWF], F32, tag="t2", name=f"t2_{n}")
                t2s[n] = t2

                def evict2(c, ps):
                    psv = ps[:].rearrange("p (r w) -> p r w", w=PITCH)[:, :, 1:1 + W]
                    sl = slice(c * 448, (c + 1) * 448)
                    ov = t2[:, sl].rearrange("p (r w) -> p r w", w=W)
                    nc.scalar.activation(ov, psv, ACTF.Relu,
                                         bias=b2[:], scale=inv2[:])
                    if chunk_store:
                        nc.sync.dma_start(
                            out_d[n].rearrange("c h w -> c (h w)")[:, sl],
                            t2[:, sl])

                _emit_conv(psum2_p, w2k, mq_pads[n % 2][:], evict2,
                           res_pad=xb_pads[n % 2][:])

            def final(n):
                nc.sync.dma_start(out_d[n].rearrange("c h w -> c (h w)"),
                                  t2s[n][:])

            # ---- software pipeline ----
            w1k = _setup_w(0)
            (((inv1, b1), (inv2, b2)), resw) = _setup_bn()
            quant1(0)
            w2k = _setup_w(1)
            quant1(1)
            conv1(0)
            quant1(2)   # after conv1(0): overwrites xq_pads[0]
            quant2(0)
            conv1(1)
            for k in range(nimg):
                conv2(k, chunk_store=(k == nimg - 1))
                if k < nimg - 1:
                    final(k)
                if k + 1 < nimg:
                    quant2(k + 1)
                if k + 2 < nimg:
                    conv1(k + 2)
                if k + 3 < nimg:
                    quant1(k + 3)
                if k + 4 < nimg:
                    load_x(k + 4)

    nc.compile()
    return nc


@lru_cache(maxsize=1)
def _get_nc():
    return build_nc(NIMG)


def kernel(x, w1, w2, gamma1, beta1, mean1, var1,
           gamma2, beta2, mean2, var2, _trace=False):
    f = lambda a: np.ascontiguousarray(np.asarray(a, dtype=np.float32))
    x = f(x)
    n_total = x.shape[0]
    assert n_total == N_CORES * NIMG, x.shape
    xs = x.reshape(N_CORES, NIMG, P, H, W)
    rep = {
        "w1": f(w1), "w2": f(w2),
        "gamma1": f(gamma1), "beta1": f(beta1), "mean1": f(mean1), "var1": f(var1),
        "gamma2": f(gamma2), "beta2": f(beta2), "mean2": f(mean2), "var2": f(var2),
    }
    in_maps = [{"x": np.ascontiguousarray(xs[c]), **rep} for c in range(N_CORES)]
    nc = _get_nc()
    res = run_bass_kernel_spmd(nc, in_maps, core_ids=list(range(N_CORES)),
                               trace=_trace)
    out = np.concatenate([res.results[c]["out"] for c in range(N_CORES)], axis=0)
    if _trace:
        kernel.last_result = res
    return out.reshape(n_total, P, H, W)
